# revision 11
# baseline (speedup 1.0000x reference)
"""2-layer GCN (GCNConv -> ReLU -> GCNConv -> log_softmax) on 8 Trainium2 cores.

Sharding: nodes are relabeled (sorted by in-degree) and dealt out in blocks of
128 round-robin across the 8 cores (graph/data parallel, as per the node-
sharding hint).  Each core:
  - transforms its node slice (x @ W1) on the tensor engine,
  - computes degrees / dinv from its in-edge weights on the vector engine,
  - AllGathers the dinv-scaled hidden table,
  - aggregates messages for its nodes (indirect-DMA row gathers + vector-
    engine weighted reduction),
  - repeats for layer 2 (aggregate-then-transform, which commutes),
  - finishes with W2 transform + log_softmax per 128-node block.
Weight matrices are replicated; outputs are gathered/unpermuted on the host.
"""

import numpy as np

N_NODES = 100000
N_EDGES = 1600000
F_IN = 500
F_PAD = 512
H = 64
C = 16
N_CORES = 8
P = 128
NPAD = 100352          # 8 * 12544 node slots after padding
ND = NPAD - N_NODES    # dummy slots (placed first: lowest degree)
NB = NPAD // (P * N_CORES)   # 98 local blocks per core
NLOC = NB * P                # 12544 local node slots per core

_PROG_CACHE = {}
PROFILE = False
LAST_EXEC_NS = None


def _build_program(T, d_loc, coffs, dmax):
    import concourse.bacc as bacc
    import concourse.bass as bass
    import concourse.mybir as mybir
    from concourse.tile import TileContext
    from concourse.tile_rust import add_dep_helper
    from concourse.masks import make_identity
    from contextlib import ExitStack

    dt = mybir.dt.float32
    nc = bacc.Bacc("TRN2", target_bir_lowering=False, debug=False,
                   num_devices=N_CORES)

    xT = nc.dram_tensor("xT", [F_PAD, NLOC], dt, kind="ExternalInput")
    W1p = nc.dram_tensor("W1p", [F_PAD, H], dt, kind="ExternalInput")
    b1t = nc.dram_tensor("b1t", [P, H], dt, kind="ExternalInput")
    W2t = nc.dram_tensor("W2t", [H, C], dt, kind="ExternalInput")
    b2t = nc.dram_tensor("b2t", [P, C], dt, kind="ExternalInput")
    wsl = nc.dram_tensor("wsl", [P, T], dt, kind="ExternalInput")
    isl = nc.dram_tensor("isl", [P, T], mybir.dt.int32, kind="ExternalInput")
    outd = nc.dram_tensor("outd", [NLOC, C], dt, kind="ExternalOutput")

    # gather tables: dedicated DRAM tensors (offset-0 requirement of
    # indirect_dma_start sources)
    hs1_loc = nc.dram_tensor("hs1_loc", [NLOC, H], dt)
    hs1_full = nc.dram_tensor("hs1_full", [NPAD, H], dt, addr_space="Shared")
    g1_loc = nc.dram_tensor("g1_loc", [NLOC, H], dt)
    g1_full = nc.dram_tensor("g1_full", [NPAD, H], dt, addr_space="Shared")

    SG = 14           # blocks per transform supergroup
    NSG = NB // SG    # 7
    HB = NB // 2
    HROWS = N_CORES * HB * P

    def bcast_inner(ap, n):
        # append a step-0 inner dim of size n to an AP view
        return bass.AP(ap.tensor, ap.offset, list(ap.ap) + [[0, n]])

    def swap_last2(ap):
        a = list(ap.ap)
        a[-1], a[-2] = a[-2], a[-1]
        return bass.AP(ap.tensor, ap.offset, a)

    with TileContext(nc) as tc, ExitStack() as ctx:
        cp = ctx.enter_context(tc.tile_pool(name="const", bufs=1))
        xp = ctx.enter_context(tc.tile_pool(name="xsg", bufs=3))
        hp = ctx.enter_context(tc.tile_pool(name="hrow", bufs=8))
        mp = ctx.enter_context(tc.tile_pool(name="msg", bufs=6))
        ap_ = ctx.enter_context(tc.tile_pool(name="agg", bufs=4))
        pp = ctx.enter_context(tc.tile_pool(name="ps", bufs=4, space="PSUM"))
        pt = ctx.enter_context(tc.tile_pool(name="pstr", bufs=2, space="PSUM"))
        po = ctx.enter_context(tc.tile_pool(name="pso", bufs=2, space="PSUM"))
        sp = ctx.enter_context(tc.tile_pool(name="small", bufs=8))

        # ---- constants ----
        w1t = cp.tile([P, 4, H], dt)
        for k in range(4):
            nc.sync.dma_start(w1t[:, k, :], W1p[k * P:(k + 1) * P, :])
        b1r = cp.tile([P, H], dt)
        nc.sync.dma_start(b1r[:], b1t[:])
        w2 = cp.tile([H, C], dt)
        nc.sync.dma_start(w2[:], W2t[:])
        b2r = cp.tile([P, C], dt)
        nc.sync.dma_start(b2r[:], b2t[:])
        wslt = cp.tile([P, T], dt)
        nc.sync.dma_start(wslt[:], wsl[:])
        islt = cp.tile([P, T], mybir.dt.int32)
        nc.sync.dma_start(islt[:], isl[:])
        ident = cp.tile([P, P], dt)
        make_identity(nc, ident[:])

        # ---- degrees -> dinv = sqrt(1/deg) ----
        deg = cp.tile([P, NB], dt)
        for b in range(NB):
            nc.vector.reduce_sum(deg[:, b:b + 1],
                                 wslt[:, coffs[b]:coffs[b] + d_loc[b]],
                                 axis=mybir.AxisListType.X)
        rdeg = cp.tile([P, NB], dt)
        nc.vector.reciprocal(rdeg[:], deg[:])
        dinv = cp.tile([P, NB], dt)
        nc.scalar.activation(dinv[:], rdeg[:],
                             mybir.ActivationFunctionType.Sqrt)

        # ---- transform: t1 = x @ W1 ; hs1 = dinv * t1 ----
        for sg in range(NSG):
            c0 = sg * SG * P
            xk = xp.tile([P, 4, SG * P], dt, tag="xk")
            for k in range(4):
                nc.sync.dma_start(xk[:, k, :], xT[k * P:(k + 1) * P, c0:c0 + SG * P])
            for bl in range(SG):
                b = sg * SG + bl
                ps = pp.tile([P, H], dt, tag="pst")
                for k in range(4):
                    nc.tensor.matmul(ps[:], lhsT=xk[:, k, bl * P:(bl + 1) * P],
                                     rhs=w1t[:, k, :],
                                     start=(k == 0), stop=(k == 3))
                hrow = hp.tile([P, H], dt, tag="hs")
                nc.vector.tensor_scalar(hrow[:], ps[:], dinv[:, b:b + 1], None,
                                        op0=mybir.AluOpType.mult)
                nc.sync.dma_start(hs1_loc[b * P:(b + 1) * P, :], hrow[:])
            if (sg + 1) * SG >= HB and sg * SG < HB:
                # first half of the local slice is complete -> overlap its
                # AllGather with the rest of the transform
                nc.gpsimd.collective_compute(
                    "AllGather", mybir.AluOpType.bypass,
                    replica_groups=[list(range(N_CORES))],
                    ins=[hs1_loc[0:HB * P, :]], outs=[hs1_full[0:HROWS, :]])

        nc.gpsimd.collective_compute(
            "AllGather", mybir.AluOpType.bypass,
            replica_groups=[list(range(N_CORES))],
            ins=[hs1_loc[HB * P:NLOC, :]], outs=[hs1_full[HROWS:NPAD, :]])

        # ---- aggregation layers ----
        def agg_layer(table, post, blo, bhi):
            for b in range(blo, bhi):
                db = d_loc[b]
                msg = mp.tile([P, dmax, H], dt, tag="msg")
                for j in range(db):
                    g = nc.gpsimd.indirect_dma_start(
                        out=msg[:, j, :], out_offset=None, in_=table[:],
                        in_offset=bass.IndirectOffsetOnAxis(
                            ap=islt[:, coffs[b] + j:coffs[b] + j + 1], axis=0))
                # weighted sum over the db in-edge slots
                wv = bcast_inner(wslt[:, coffs[b]:coffs[b] + db], H)
                nc.vector.tensor_tensor(out=msg[:, :db, :], in0=msg[:, :db, :],
                                        in1=wv, op=mybir.AluOpType.mult)
                agg = ap_.tile([P, H], dt, tag="agg")
                nc.vector.reduce_sum(agg[:], swap_last2(msg[:, :db, :]),
                                     axis=mybir.AxisListType.X)
                post(b, agg)

        # layer 1 post: h1 = relu(dinv*agg + b1) ; g1 = dinv*h1
        def post1(b, agg):
            nc.vector.tensor_scalar(agg[:], agg[:], dinv[:, b:b + 1], None,
                                    op0=mybir.AluOpType.mult)
            nc.vector.tensor_tensor(out=agg[:], in0=agg[:],
                                    in1=b1r[:],
                                    op=mybir.AluOpType.add)
            nc.vector.tensor_scalar_max(agg[:], agg[:], 0.0)
            g1row = hp.tile([P, H], dt, tag="g1")
            nc.vector.tensor_scalar(g1row[:], agg[:], dinv[:, b:b + 1], None,
                                    op0=mybir.AluOpType.mult)
            nc.sync.dma_start(g1_loc[b * P:(b + 1) * P, :], g1row[:])

        agg_layer(hs1_full, post1, 0, HB)
        # first half of g1 done -> overlap its AllGather with the second half
        nc.gpsimd.collective_compute(
            "AllGather", mybir.AluOpType.bypass,
            replica_groups=[list(range(N_CORES))],
            ins=[g1_loc[0:HB * P, :]], outs=[g1_full[0:HROWS, :]])
        agg_layer(hs1_full, post1, HB, NB)
        nc.gpsimd.collective_compute(
            "AllGather", mybir.AluOpType.bypass,
            replica_groups=[list(range(N_CORES))],
            ins=[g1_loc[HB * P:NLOC, :]], outs=[g1_full[HROWS:NPAD, :]])

        # layer 2 post: sc2 = dinv*agg2 ; out = log_softmax(sc2 @ W2 + b2)
        def post2(b, agg):
            nc.vector.tensor_scalar(agg[:], agg[:], dinv[:, b:b + 1], None,
                                    op0=mybir.AluOpType.mult)
            ptr = pt.tile([H, P], dt, tag="ptr")
            nc.tensor.transpose(ptr[:], agg[:], ident[:])
            scT = sp.tile([H, P], dt, tag="scT")
            nc.vector.tensor_copy(scT[:], ptr[:])
            pso = po.tile([P, C], dt, tag="pso")
            nc.tensor.matmul(pso[:], lhsT=scT[:], rhs=w2[:],
                             start=True, stop=True)
            o = sp.tile([P, C], dt, tag="o")
            nc.vector.tensor_tensor(out=o[:], in0=pso[:],
                                    in1=b2r[:],
                                    op=mybir.AluOpType.add)
            negm = sp.tile([P, 1], dt, tag="negm")
            nc.vector.tensor_reduce(negm[:], o[:], axis=mybir.AxisListType.X,
                                    op=mybir.AluOpType.max, negate=True)
            e = sp.tile([P, C], dt, tag="e")
            s = sp.tile([P, 1], dt, tag="s")
            nc.scalar.activation(e[:], o[:], mybir.ActivationFunctionType.Exp,
                                 bias=negm[:], accum_out=s[:])
            ls = sp.tile([P, 1], dt, tag="ls")
            nc.scalar.activation(ls[:], s[:], mybir.ActivationFunctionType.Ln)
            nmls = sp.tile([P, 1], dt, tag="nmls")
            nc.vector.tensor_tensor(out=nmls[:], in0=negm[:], in1=ls[:],
                                    op=mybir.AluOpType.subtract)
            ob = sp.tile([P, C], dt, tag="ob")
            nc.vector.tensor_scalar(ob[:], o[:], nmls[:], None,
                                    op0=mybir.AluOpType.add)
            nc.sync.dma_start(outd[b * P:(b + 1) * P, :], ob[:])

        agg_layer(g1_full, post2, 0, NB)

    nc.compile()
    return nc


def _prep(x, edge_index, edge_weight, W1, b1, W2, b2):
    x = np.asarray(x, dtype=np.float32)
    ei = np.asarray(edge_index).astype(np.int64)
    ew = np.asarray(edge_weight, dtype=np.float32)
    W1 = np.asarray(W1, dtype=np.float32)
    b1 = np.asarray(b1, dtype=np.float32)
    W2 = np.asarray(W2, dtype=np.float32)
    b2 = np.asarray(b2, dtype=np.float32)

    # self loops
    rows = np.concatenate([ei[0], np.arange(N_NODES, dtype=np.int64)])
    cols = np.concatenate([ei[1], np.arange(N_NODES, dtype=np.int64)])
    ws = np.concatenate([ew, np.ones(N_NODES, dtype=np.float32)])

    indeg = np.bincount(cols, minlength=N_NODES)
    perm = np.argsort(indeg, kind="stable")          # old ids, ascending degree
    # new slot s: s < ND -> dummy ; else real node perm[s - ND]
    new_of_old = np.empty(N_NODES, dtype=np.int64)
    new_of_old[perm] = np.arange(N_NODES, dtype=np.int64) + ND

    # slot -> (core, local block, partition) ; table row.  The table is laid
    # out in two halves (blocks 0..HB-1 core-major, then blocks HB..NB-1) so
    # each half can be AllGathered as soon as it is produced.
    HB = NB // 2
    HROWS = N_CORES * HB * P

    def table_row_of_new(s):
        kg = s // P
        p = s % P
        c = kg % N_CORES
        b = kg // N_CORES
        lo = c * HB * P + b * P + p
        hi = HROWS + c * (NB - HB) * P + (b - HB) * P + p
        return np.where(b < HB, lo, hi)

    r_new = new_of_old[rows]
    c_new = new_of_old[cols]
    kg = c_new // P
    core_of_edge = kg % N_CORES
    b_of_edge = kg // N_CORES
    p_of_edge = c_new % P
    src_row = table_row_of_new(r_new)

    # unified per-local-block chunk counts across cores
    deg_slot = np.zeros(NPAD, dtype=np.int64)
    deg_slot[ND:] = indeg[perm] + 1                  # incl self loop
    d_glob = deg_slot.reshape(-1, P).max(axis=1)
    d_loc = d_glob.reshape(NB, N_CORES).max(axis=1).astype(np.int64)
    d_loc = np.maximum(d_loc, 1)
    coffs = np.zeros(NB, dtype=np.int64)
    coffs[1:] = np.cumsum(d_loc)[:-1]
    T = int(d_loc.sum())
    dmax = int(d_loc.max())

    # slot grids per core
    wslab = np.zeros((N_CORES, P, T), dtype=np.float32)
    islab = np.zeros((N_CORES, P, T), dtype=np.int32)
    order = np.lexsort((p_of_edge, b_of_edge, core_of_edge))
    ce, be, pe = core_of_edge[order], b_of_edge[order], p_of_edge[order]
    se, we = src_row[order], ws[order]
    # j index within each (core, block, partition) group
    key = (ce * NB + be) * P + pe
    start = np.r_[True, key[1:] != key[:-1]]
    gidx = np.arange(len(key)) - np.maximum.accumulate(np.where(start, np.arange(len(key)), 0))
    colpos = coffs[be] + gidx
    wslab[ce, pe, colpos] = we
    islab[ce, pe, colpos] = se.astype(np.int32)
    # dummy partitions (no in-edges at all) would get deg=0 -> dinv=inf -> NaN
    # rows in the gather table, and pad slots gather row 0 (a dummy) with w=0,
    # making 0*NaN poison real sums.  Give every empty (core, block, partition)
    # one unit-weight slot pointing at row 0 so its deg is 1 and its table row
    # is finite (its transform output is 0, so the row stays all-zero).
    occ = np.zeros((N_CORES, P, NB), dtype=bool)
    occ[ce, pe, be] = True
    cz, pz, bz = np.nonzero(~occ)
    wslab[cz, pz, coffs[bz]] = 1.0

    # x transposed + relabeled per core
    xTs = []
    outmap = []   # per core: old node id per local slot (-1 dummy)
    for c in range(N_CORES):
        old_ids = np.full(NLOC, -1, dtype=np.int64)
        # local slot l = b*P + p  -> new id s = ((b*8 + c) * P + p)
        ls = np.arange(NLOC)
        bb, pp_ = ls // P, ls % P
        s_new = (bb * N_CORES + c) * P + pp_
        real = s_new >= ND
        old_ids[real] = perm[s_new[real] - ND]
        Xc = np.zeros((NLOC, F_PAD), dtype=np.float32)
        Xc[real, :F_IN] = x[old_ids[real]]
        xTs.append(np.ascontiguousarray(Xc.T))
        outmap.append(old_ids)

    W1p = np.zeros((F_PAD, H), dtype=np.float32)
    W1p[:F_IN] = W1
    in_maps = []
    for c in range(N_CORES):
        in_maps.append({
            "xT": xTs[c], "W1p": W1p, "b1t": np.tile(b1[None, :], (P, 1)),
            "W2t": W2.copy(), "b2t": np.tile(b2[None, :], (P, 1)),
            "wsl": wslab[c], "isl": islab[c],
        })
    return in_maps, outmap, (T, tuple(d_loc.tolist()), tuple(coffs.tolist()), dmax)


def kernel(x, edge_index, edge_weight, W1, b1, W2, b2):
    from concourse.bass_utils import run_bass_kernel_spmd

    in_maps, outmap, (T, d_loc, coffs, dmax) = _prep(
        x, edge_index, edge_weight, W1, b1, W2, b2)

    key = (T, d_loc, coffs, dmax)
    if key not in _PROG_CACHE:
        _PROG_CACHE[key] = _build_program(T, list(d_loc), list(coffs), dmax)
    nc = _PROG_CACHE[key]

    global LAST_EXEC_NS
    res = run_bass_kernel_spmd(nc, in_maps, core_ids=list(range(N_CORES)),
                               trace=PROFILE)
    if res.exec_time_ns:
        LAST_EXEC_NS = res.exec_time_ns
    out = np.zeros((N_NODES, C), dtype=np.float32)
    for c in range(N_CORES):
        oc = np.asarray(res.results[c]["outd"], dtype=np.float32)
        m = outmap[c]
        real = m >= 0
        out[m[real]] = oc[real]
    return out


# revision 14
# speedup vs baseline: 1.1638x; 1.1638x over previous
"""2-layer GCN (GCNConv -> ReLU -> GCNConv -> log_softmax) on 8 Trainium2 cores.

Sharding: nodes are relabeled (sorted by in-degree) and dealt out in blocks of
128 round-robin across the 8 cores (graph/data parallel, as per the node-
sharding hint).  Each core:
  - transforms its node slice (x @ W1) on the tensor engine,
  - computes degrees / dinv from its in-edge weights on the vector engine,
  - AllGathers the dinv-scaled hidden table,
  - aggregates messages for its nodes (indirect-DMA row gathers + vector-
    engine weighted reduction),
  - repeats for layer 2 (aggregate-then-transform, which commutes),
  - finishes with W2 transform + log_softmax per 128-node block.
Weight matrices are replicated; outputs are gathered/unpermuted on the host.
"""

import numpy as np

N_NODES = 100000
N_EDGES = 1600000
F_IN = 500
F_PAD = 512
H = 64
C = 16
N_CORES = 8
P = 128
NPAD = 100352          # 8 * 12544 node slots after padding
ND = NPAD - N_NODES    # dummy slots (placed first: lowest degree)
NB = NPAD // (P * N_CORES)   # 98 local blocks per core
NLOC = NB * P                # 12544 local node slots per core

_PROG_CACHE = {}
PROFILE = False
LAST_EXEC_NS = None


def _build_program(T, d_loc, coffs, dmax):
    import concourse.bacc as bacc
    import concourse.bass as bass
    import concourse.mybir as mybir
    from concourse.tile import TileContext
    from concourse.tile_rust import add_dep_helper
    from concourse.masks import make_identity
    from contextlib import ExitStack

    dt = mybir.dt.float32
    nc = bacc.Bacc("TRN2", target_bir_lowering=False, debug=False,
                   num_devices=N_CORES)

    xT = nc.dram_tensor("xT", [F_PAD, NLOC], dt, kind="ExternalInput")
    W1p = nc.dram_tensor("W1p", [F_PAD, H], dt, kind="ExternalInput")
    b1t = nc.dram_tensor("b1t", [P, H], dt, kind="ExternalInput")
    W2t = nc.dram_tensor("W2t", [H, C], dt, kind="ExternalInput")
    b2t = nc.dram_tensor("b2t", [P, C], dt, kind="ExternalInput")
    wsl = nc.dram_tensor("wsl", [P, T], dt, kind="ExternalInput")
    isl = nc.dram_tensor("isl", [P, T], mybir.dt.int32, kind="ExternalInput")
    outd = nc.dram_tensor("outd", [NLOC, C], dt, kind="ExternalOutput")

    # gather tables: dedicated DRAM tensors (offset-0 requirement of
    # indirect_dma_start sources)
    hs1_loc = nc.dram_tensor("hs1_loc", [NLOC, H], dt)
    hs1_full = nc.dram_tensor("hs1_full", [NPAD, H], dt, addr_space="Shared")
    g1_loc = nc.dram_tensor("g1_loc", [NLOC, H], dt)
    g1_full = nc.dram_tensor("g1_full", [NPAD, H], dt, addr_space="Shared")

    SG = 14           # blocks per transform supergroup
    NSG = NB // SG    # 7
    HB = NB // 2
    HROWS = N_CORES * HB * P

    def bcast_inner(ap, n):
        # append a step-0 inner dim of size n to an AP view
        return bass.AP(ap.tensor, ap.offset, list(ap.ap) + [[0, n]])

    def swap_last2(ap):
        a = list(ap.ap)
        a[-1], a[-2] = a[-2], a[-1]
        return bass.AP(ap.tensor, ap.offset, a)

    with TileContext(nc) as tc, ExitStack() as ctx:
        cp = ctx.enter_context(tc.tile_pool(name="const", bufs=1))
        xp = ctx.enter_context(tc.tile_pool(name="xsg", bufs=2))
        hp = ctx.enter_context(tc.tile_pool(name="hrow", bufs=8))
        mp = ctx.enter_context(tc.tile_pool(name="msg", bufs=6))
        ap_ = ctx.enter_context(tc.tile_pool(name="agg", bufs=4))
        pp = ctx.enter_context(tc.tile_pool(name="ps", bufs=4, space="PSUM"))
        pt = ctx.enter_context(tc.tile_pool(name="pstr", bufs=2, space="PSUM"))
        po = ctx.enter_context(tc.tile_pool(name="pso", bufs=2, space="PSUM"))
        sp = ctx.enter_context(tc.tile_pool(name="small", bufs=8))

        # ---- constants ----
        w1t = cp.tile([P, 4, H], dt)
        for k in range(4):
            nc.sync.dma_start(w1t[:, k, :], W1p[k * P:(k + 1) * P, :])
        b1r = cp.tile([P, H], dt)
        nc.sync.dma_start(b1r[:], b1t[:])
        w2 = cp.tile([H, C], dt)
        nc.sync.dma_start(w2[:], W2t[:])
        b2r = cp.tile([P, C], dt)
        nc.sync.dma_start(b2r[:], b2t[:])
        wslt = cp.tile([P, T], dt)
        nc.sync.dma_start(wslt[:], wsl[:])
        islt = cp.tile([P, T], mybir.dt.int32)
        nc.sync.dma_start(islt[:], isl[:])
        ident = cp.tile([P, P], dt)
        make_identity(nc, ident[:])

        # ---- degrees -> dinv = sqrt(1/deg) ----
        deg = cp.tile([P, NB], dt)
        for b in range(NB):
            nc.vector.reduce_sum(deg[:, b:b + 1],
                                 wslt[:, coffs[b]:coffs[b] + d_loc[b]],
                                 axis=mybir.AxisListType.X)
        rdeg = cp.tile([P, NB], dt)
        nc.vector.reciprocal(rdeg[:], deg[:])
        dinv = cp.tile([P, NB], dt)
        nc.scalar.activation(dinv[:], rdeg[:],
                             mybir.ActivationFunctionType.Sqrt)

        # ---- transform: t1 = x @ W1 ; hs1 = dinv * t1 ----
        for sg in range(NSG):
            c0 = sg * SG * P
            xk = xp.tile([P, 4, SG * P], dt, tag="xk")
            for k in range(4):
                nc.sync.dma_start(xk[:, k, :], xT[k * P:(k + 1) * P, c0:c0 + SG * P])
            for bl in range(SG):
                b = sg * SG + bl
                ps = pp.tile([P, H], dt, tag="pst")
                for k in range(4):
                    nc.tensor.matmul(ps[:], lhsT=xk[:, k, bl * P:(bl + 1) * P],
                                     rhs=w1t[:, k, :],
                                     start=(k == 0), stop=(k == 3))
                hrow = hp.tile([P, H], dt, tag="hs")
                nc.vector.tensor_scalar(hrow[:], ps[:], dinv[:, b:b + 1], None,
                                        op0=mybir.AluOpType.mult)
                nc.sync.dma_start(hs1_loc[b * P:(b + 1) * P, :], hrow[:])
            if (sg + 1) * SG >= HB and sg * SG < HB:
                # first half of the local slice is complete -> overlap its
                # AllGather with the rest of the transform
                nc.gpsimd.collective_compute(
                    "AllGather", mybir.AluOpType.bypass,
                    replica_groups=[list(range(N_CORES))],
                    ins=[hs1_loc[0:HB * P, :]], outs=[hs1_full[0:HROWS, :]])

        nc.gpsimd.collective_compute(
            "AllGather", mybir.AluOpType.bypass,
            replica_groups=[list(range(N_CORES))],
            ins=[hs1_loc[HB * P:NLOC, :]], outs=[hs1_full[HROWS:NPAD, :]])

        # ---- aggregation layers ----
        def agg_layer(table, post, blo, bhi):
            for b in range(blo, bhi):
                db = d_loc[b]
                msg = mp.tile([P, dmax, H], dt, tag="msg")
                for j in range(db):
                    g = nc.gpsimd.indirect_dma_start(
                        out=msg[:, j, :], out_offset=None, in_=table[:],
                        in_offset=bass.IndirectOffsetOnAxis(
                            ap=islt[:, coffs[b] + j:coffs[b] + j + 1], axis=0))
                # weighted sum over the db in-edge slots
                wv = bcast_inner(wslt[:, coffs[b]:coffs[b] + db], H)
                nc.vector.tensor_tensor(out=msg[:, :db, :], in0=msg[:, :db, :],
                                        in1=wv, op=mybir.AluOpType.mult)
                agg = ap_.tile([P, H], dt, tag="agg")
                nc.vector.reduce_sum(agg[:], swap_last2(msg[:, :db, :]),
                                     axis=mybir.AxisListType.X)
                post(b, agg)

        # layer 1 post: h1 = relu(dinv*agg + b1) ; g1 = dinv*h1
        def post1(b, agg):
            nc.vector.tensor_scalar(agg[:], agg[:], dinv[:, b:b + 1], None,
                                    op0=mybir.AluOpType.mult)
            nc.vector.tensor_tensor(out=agg[:], in0=agg[:],
                                    in1=b1r[:],
                                    op=mybir.AluOpType.add)
            nc.vector.tensor_scalar_max(agg[:], agg[:], 0.0)
            g1row = hp.tile([P, H], dt, tag="g1")
            nc.vector.tensor_scalar(g1row[:], agg[:], dinv[:, b:b + 1], None,
                                    op0=mybir.AluOpType.mult)
            nc.sync.dma_start(g1_loc[b * P:(b + 1) * P, :], g1row[:])

        agg_layer(hs1_full, post1, 0, HB)
        # first half of g1 done -> overlap its AllGather with the second half
        nc.gpsimd.collective_compute(
            "AllGather", mybir.AluOpType.bypass,
            replica_groups=[list(range(N_CORES))],
            ins=[g1_loc[0:HB * P, :]], outs=[g1_full[0:HROWS, :]])
        agg_layer(hs1_full, post1, HB, NB)
        nc.gpsimd.collective_compute(
            "AllGather", mybir.AluOpType.bypass,
            replica_groups=[list(range(N_CORES))],
            ins=[g1_loc[HB * P:NLOC, :]], outs=[g1_full[HROWS:NPAD, :]])

        # layer 2 post: sc2 = dinv*agg2 ; out = log_softmax(sc2 @ W2 + b2)
        def post2(b, agg):
            nc.vector.tensor_scalar(agg[:], agg[:], dinv[:, b:b + 1], None,
                                    op0=mybir.AluOpType.mult)
            ptr = pt.tile([H, P], dt, tag="ptr")
            nc.tensor.transpose(ptr[:], agg[:], ident[:])
            scT = sp.tile([H, P], dt, tag="scT")
            nc.vector.tensor_copy(scT[:], ptr[:])
            pso = po.tile([P, C], dt, tag="pso")
            nc.tensor.matmul(pso[:], lhsT=scT[:], rhs=w2[:],
                             start=True, stop=True)
            o = sp.tile([P, C], dt, tag="o")
            nc.vector.tensor_tensor(out=o[:], in0=pso[:],
                                    in1=b2r[:],
                                    op=mybir.AluOpType.add)
            negm = sp.tile([P, 1], dt, tag="negm")
            nc.vector.tensor_reduce(negm[:], o[:], axis=mybir.AxisListType.X,
                                    op=mybir.AluOpType.max, negate=True)
            e = sp.tile([P, C], dt, tag="e")
            s = sp.tile([P, 1], dt, tag="s")
            nc.scalar.activation(e[:], o[:], mybir.ActivationFunctionType.Exp,
                                 bias=negm[:], accum_out=s[:])
            ls = sp.tile([P, 1], dt, tag="ls")
            nc.scalar.activation(ls[:], s[:], mybir.ActivationFunctionType.Ln)
            nmls = sp.tile([P, 1], dt, tag="nmls")
            nc.vector.tensor_tensor(out=nmls[:], in0=negm[:], in1=ls[:],
                                    op=mybir.AluOpType.subtract)
            ob = sp.tile([P, C], dt, tag="ob")
            nc.vector.tensor_scalar(ob[:], o[:], nmls[:], None,
                                    op0=mybir.AluOpType.add)
            nc.sync.dma_start(outd[b * P:(b + 1) * P, :], ob[:])

        agg_layer(g1_full, post2, 0, NB)

    nc.compile()
    return nc


def _prep(x, edge_index, edge_weight, W1, b1, W2, b2):
    x = np.asarray(x, dtype=np.float32)
    ei = np.asarray(edge_index).astype(np.int64)
    ew = np.asarray(edge_weight, dtype=np.float32)
    W1 = np.asarray(W1, dtype=np.float32)
    b1 = np.asarray(b1, dtype=np.float32)
    W2 = np.asarray(W2, dtype=np.float32)
    b2 = np.asarray(b2, dtype=np.float32)

    # self loops
    rows = np.concatenate([ei[0], np.arange(N_NODES, dtype=np.int64)])
    cols = np.concatenate([ei[1], np.arange(N_NODES, dtype=np.int64)])
    ws = np.concatenate([ew, np.ones(N_NODES, dtype=np.float32)])

    indeg = np.bincount(cols, minlength=N_NODES)
    perm = np.argsort(indeg, kind="stable")          # old ids, ascending degree
    # new slot s: s < ND -> dummy ; else real node perm[s - ND]
    new_of_old = np.empty(N_NODES, dtype=np.int64)
    new_of_old[perm] = np.arange(N_NODES, dtype=np.int64) + ND

    # slot -> (core, local block, partition) ; table row.  The table is laid
    # out in two halves (blocks 0..HB-1 core-major, then blocks HB..NB-1) so
    # each half can be AllGathered as soon as it is produced.
    HB = NB // 2
    HROWS = N_CORES * HB * P

    def table_row_of_new(s):
        kg = s // P
        p = s % P
        c = kg % N_CORES
        b = kg // N_CORES
        lo = c * HB * P + b * P + p
        hi = HROWS + c * (NB - HB) * P + (b - HB) * P + p
        return np.where(b < HB, lo, hi)

    r_new = new_of_old[rows]
    c_new = new_of_old[cols]
    kg = c_new // P
    core_of_edge = kg % N_CORES
    b_of_edge = kg // N_CORES
    p_of_edge = c_new % P
    src_row = table_row_of_new(r_new)

    # unified per-local-block chunk counts across cores
    deg_slot = np.zeros(NPAD, dtype=np.int64)
    deg_slot[ND:] = indeg[perm] + 1                  # incl self loop
    d_glob = deg_slot.reshape(-1, P).max(axis=1)
    d_loc = d_glob.reshape(NB, N_CORES).max(axis=1).astype(np.int64)
    d_loc = np.maximum(d_loc, 1)
    coffs = np.zeros(NB, dtype=np.int64)
    coffs[1:] = np.cumsum(d_loc)[:-1]
    T = int(d_loc.sum())
    dmax = int(d_loc.max())

    # slot grids per core
    wslab = np.zeros((N_CORES, P, T), dtype=np.float32)
    islab = np.zeros((N_CORES, P, T), dtype=np.int32)
    order = np.lexsort((p_of_edge, b_of_edge, core_of_edge))
    ce, be, pe = core_of_edge[order], b_of_edge[order], p_of_edge[order]
    se, we = src_row[order], ws[order]
    # j index within each (core, block, partition) group
    key = (ce * NB + be) * P + pe
    start = np.r_[True, key[1:] != key[:-1]]
    gidx = np.arange(len(key)) - np.maximum.accumulate(np.where(start, np.arange(len(key)), 0))
    colpos = coffs[be] + gidx
    wslab[ce, pe, colpos] = we
    islab[ce, pe, colpos] = se.astype(np.int32)
    # dummy partitions (no in-edges at all) would get deg=0 -> dinv=inf -> NaN
    # rows in the gather table, and pad slots gather row 0 (a dummy) with w=0,
    # making 0*NaN poison real sums.  Give every empty (core, block, partition)
    # one unit-weight slot pointing at row 0 so its deg is 1 and its table row
    # is finite (its transform output is 0, so the row stays all-zero).
    occ = np.zeros((N_CORES, P, NB), dtype=bool)
    occ[ce, pe, be] = True
    cz, pz, bz = np.nonzero(~occ)
    wslab[cz, pz, coffs[bz]] = 1.0

    # x transposed + relabeled per core
    xTs = []
    outmap = []   # per core: old node id per local slot (-1 dummy)
    for c in range(N_CORES):
        old_ids = np.full(NLOC, -1, dtype=np.int64)
        # local slot l = b*P + p  -> new id s = ((b*8 + c) * P + p)
        ls = np.arange(NLOC)
        bb, pp_ = ls // P, ls % P
        s_new = (bb * N_CORES + c) * P + pp_
        real = s_new >= ND
        old_ids[real] = perm[s_new[real] - ND]
        Xc = np.zeros((NLOC, F_PAD), dtype=np.float32)
        Xc[real, :F_IN] = x[old_ids[real]]
        xTs.append(np.ascontiguousarray(Xc.T))
        outmap.append(old_ids)

    W1p = np.zeros((F_PAD, H), dtype=np.float32)
    W1p[:F_IN] = W1
    in_maps = []
    for c in range(N_CORES):
        in_maps.append({
            "xT": xTs[c], "W1p": W1p, "b1t": np.tile(b1[None, :], (P, 1)),
            "W2t": W2.copy(), "b2t": np.tile(b2[None, :], (P, 1)),
            "wsl": wslab[c], "isl": islab[c],
        })
    return in_maps, outmap, (T, tuple(d_loc.tolist()), tuple(coffs.tolist()), dmax)


def kernel(x, edge_index, edge_weight, W1, b1, W2, b2):
    from concourse.bass_utils import run_bass_kernel_spmd

    in_maps, outmap, (T, d_loc, coffs, dmax) = _prep(
        x, edge_index, edge_weight, W1, b1, W2, b2)

    key = (T, d_loc, coffs, dmax)
    if key not in _PROG_CACHE:
        _PROG_CACHE[key] = _build_program(T, list(d_loc), list(coffs), dmax)
    nc = _PROG_CACHE[key]

    global LAST_EXEC_NS
    res = run_bass_kernel_spmd(nc, in_maps, core_ids=list(range(N_CORES)),
                               trace=PROFILE)
    if res.exec_time_ns:
        LAST_EXEC_NS = res.exec_time_ns
    out = np.zeros((N_NODES, C), dtype=np.float32)
    for c in range(N_CORES):
        oc = np.asarray(res.results[c]["outd"], dtype=np.float32)
        m = outmap[c]
        real = m >= 0
        out[m[real]] = oc[real]
    return out
